# revision 9
# baseline (speedup 1.0000x reference)
"""Bass/Tile TRN2 kernel for nn_BiStochastic (batched Sinkhorn, 10 iters).

Math: every iteration only rescales rows or columns, so
    s_final = diag(r) @ s0 @ diag(c)
with r, c [512]-vectors evolved by alternating matvecs:
    col step (even i): u = s0^T r ; c <- c * 1/(c*u)
    row step (odd  i): v = s0 c   ; r <- r * 1/(r*v + eps)
Steps 0..8 run as PE matvecs over bf16 copies of s0 / s0^T (errors from the
low-precision stats are contracted away by subsequent iterations; measured
~3e-4 max rel err). Step 9 (the last row step) is computed exactly in fp32,
fused into the final scaling pass via scalar_tensor_tensor(accum_out=rowsum).

PSUM tiles are allocated statically (full-bank, manual parity double buffer):
lazily-reused PSUM banks across kinds make Tile emit cross-tile WAW waits on
matmuls, and the MM ISA slot allows a single sync wait. With pinned banks and
one reader-engine per PSUM tensor every matmul needs at most one wait.

Sharding: pure data parallel, batch 256 -> 32 matrices per core x 8 cores.
"""

import sys

sys.path.insert(0, "/opt/trn_rl_repo")

import numpy as np

import concourse.bacc as bacc
import concourse.mybir as mybir
import concourse.tile as tile
from concourse import masks
from concourse.bass_utils import run_bass_kernel_spmd

N_CORES = 8
B_SHARD = 32  # 256 / 8
N = 512
P = 128
NCH = N // P  # 4 chunks of 128
EPS = 1e-4
F32 = mybir.dt.float32
BF16 = mybir.dt.bfloat16

MUL = mybir.AluOpType.mult


def build_program():
    nc = bacc.Bacc()
    s_in = nc.declare_dram_parameter("s", [B_SHARD, N, N], F32, isOutput=False)
    s_out = nc.declare_dram_parameter("out", [B_SHARD, N, N], F32, isOutput=True)

    with tile.TileContext(nc) as tc:
        with (
            tc.tile_pool(name="singles", bufs=1) as singles,
            tc.tile_pool(name="data", bufs=3) as data,
            tc.tile_pool(name="vec", bufs=4) as vec,
            tc.tile_pool(name="psum_fix", bufs=1, space="PSUM") as psum_fix,
        ):
            ident_b = singles.tile([P, P], BF16)
            masks.make_identity(nc, ident_b[:])
            one11 = singles.tile([1, 1], F32)
            nc.gpsimd.memset(one11[:], 1.0)
            ones_row = singles.tile([1, P], F32)
            nc.gpsimd.memset(ones_row[:], 1.0)

            # Statically pinned PSUM, one full bank per tile. Writers: PE.
            # Readers: mv/tr -> ACT only; stg/bc -> DVE only.
            mv2 = [
                psum_fix.tile([P, N], F32, tag=f"mv{i}", name=f"mv{i}")
                for i in range(2)
            ]
            tr2 = [
                psum_fix.tile([P, N], F32, tag=f"tr{i}", name=f"tr{i}")
                for i in range(2)
            ]
            stg2 = [
                psum_fix.tile([P, 2 * N], BF16, tag=f"stg{i}", name=f"stg{i}")
                for i in range(2)
            ]
            bc = psum_fix.tile([P, N], F32, tag="bc")

            for b in range(B_SHARD):
                # ---- load matrix: 4 row-chunks [128, 512] side by side ----
                sf = data.tile([P, NCH, N], F32, tag="sf")
                dram_view = s_in[b].rearrange("(t p) n -> p t n", p=P)
                nc.sync.dma_start(sf[:], dram_view)

                # bf16 copy (natural layout) for u-step matvecs
                sb = data.tile([P, NCH, N], BF16, tag="sb")
                nc.vector.tensor_copy(sb[:], sf[:])

                # bf16 transposed copy for v-step matvecs:
                # st[p, u, i] = s0[i, 128u + p]
                st = data.tile([P, NCH, N], BF16, tag="st")
                for u in range(NCH):
                    stg = stg2[u % 2]
                    for t in range(NCH):
                        nc.tensor.transpose(
                            stg[:, t * P : (t + 1) * P],
                            sb[:, t, u * P : (u + 1) * P],
                            ident_b[:],
                        )
                    nc.vector.tensor_copy(st[:, u, :], stg[:, :N])

                # ---- scaling-vector state ----
                r = vec.tile([P, NCH], F32, tag="r")
                c = vec.tile([P, NCH], F32, tag="c")
                rb = vec.tile([P, NCH], BF16, tag="rb")
                cb = vec.tile([P, NCH], BF16, tag="cb")
                nc.gpsimd.memset(r[:], 1.0)
                nc.gpsimd.memset(c[:], 1.0)
                nc.gpsimd.memset(rb[:], 1.0)

                for step in range(9):
                    # matvec on PE: [1,512] psum accumulated over 4 k-chunks
                    mv = mv2[step % 2][:1, :]
                    for m in range(NCH):
                        if step % 2 == 0:
                            lhsT, rhs = rb[:, m : m + 1], sb[:, m, :]
                        else:
                            lhsT, rhs = cb[:, m : m + 1], st[:, m, :]
                        nc.tensor.matmul(
                            mv, lhsT, rhs, start=(m == 0), stop=(m == NCH - 1)
                        )
                    urow = vec.tile([1, N], F32, tag="urow")
                    nc.scalar.copy(urow[:], mv)

                    if step == 8:
                        # final col step: c = 1/u exactly (c-factor cancels);
                        # computed in row layout, ready for the broadcast below
                        c8row = vec.tile([1, N], F32, tag="c8row")
                        nc.vector.reciprocal(c8row[:], urow[:])
                        break

                    # row [1,512] -> chunks [128,4] via tiny PE transposes
                    tr = tr2[step % 2]
                    for m in range(NCH):
                        nc.tensor.transpose(
                            tr[:, m : m + 1], urow[:, m * P : (m + 1) * P], one11[:]
                        )
                    uv = vec.tile([P, NCH], F32, tag="uv")
                    nc.scalar.copy(uv[:], tr[:, :NCH])

                    t0 = vec.tile([P, NCH], F32, tag="t0")
                    if step % 2 == 0:  # col update: c *= 1/(c*u)
                        nc.vector.tensor_mul(t0[:], c[:], uv[:])
                        nc.vector.reciprocal(t0[:], t0[:])
                        nc.vector.tensor_mul(c[:], c[:], t0[:])
                        nc.vector.tensor_copy(cb[:], c[:])
                    else:  # row update: r *= 1/(r*v + eps)
                        nc.vector.tensor_mul(t0[:], r[:], uv[:])
                        nc.vector.tensor_scalar_add(t0[:], t0[:], EPS)
                        nc.vector.reciprocal(t0[:], t0[:])
                        nc.vector.tensor_mul(r[:], r[:], t0[:])
                        nc.vector.tensor_copy(rb[:], r[:])

                # ---- broadcast final c along partitions: bc[p, j] = c_j ----
                nc.tensor.matmul(bc[:], ones_row[:], c8row[:], start=True, stop=True)

                # ---- final pass: m = (r * s0) * c  with exact rowsums fused ----
                w = vec.tile([P, NCH], F32, tag="w")
                for t in range(NCH):
                    nc.vector.scalar_tensor_tensor(
                        out=sf[:, t, :],
                        in0=sf[:, t, :],
                        scalar=r[:, t : t + 1],
                        in1=bc[:],
                        op0=MUL,
                        op1=MUL,
                        accum_out=w[:, t : t + 1],
                    )
                rr = vec.tile([P, NCH], F32, tag="rr")
                nc.vector.tensor_scalar_add(rr[:], w[:], EPS)
                nc.vector.reciprocal(rr[:], rr[:])
                for t in range(NCH):
                    nc.gpsimd.tensor_scalar_mul(sf[:, t, :], sf[:, t, :], rr[:, t : t + 1])

                out_view = s_out[b].rearrange("(t p) n -> p t n", p=P)
                nc.sync.dma_start(out_view, sf[:])

    nc.compile()
    return nc


_PROGRAM = None


def _get_program():
    global _PROGRAM
    if _PROGRAM is None:
        _PROGRAM = build_program()
    return _PROGRAM


def kernel(**inputs):
    s = np.asarray(inputs["s"], dtype=np.float32)
    assert s.shape == (N_CORES * B_SHARD, N, N), s.shape
    nc = _get_program()
    in_maps = [
        {"s": np.ascontiguousarray(s[i * B_SHARD : (i + 1) * B_SHARD])}
        for i in range(N_CORES)
    ]
    res = run_bass_kernel_spmd(nc, in_maps, core_ids=list(range(N_CORES)))
    out = np.concatenate([res.results[i]["out"] for i in range(N_CORES)], axis=0)
    return out.astype(np.float32)


if __name__ == "__main__":
    rng = np.random.default_rng(0)
    s = rng.random((N_CORES * B_SHARD, N, N), dtype=np.float32)
    o = kernel(s=s)
    print(o.shape, o.dtype)


# revision 17
# speedup vs baseline: 12069.8203x; 12069.8203x over previous
"""Bass/Tile TRN2 kernel for nn_BiStochastic (batched Sinkhorn, 10 iters).

Math: every iteration only rescales rows or columns, so
    s_final = diag(r) @ s0 @ diag(c)
with r, c [512]-vectors evolved by alternating matvecs on the PE:
    col step (even i): u = s0^T r ; c <- 1/u            (exact cancellation)
    row step (odd  i): v = s0 c   ; q <- eps*q + v ; r = 1/q   (q = 1/r)
Steps 0..7 use fp8e4m3 copies of s0 / s0^T with DoubleRow matmuls (2 k-chunks
per MM at 0.5 cyc/row); step 8 uses a bf16 copy. Low-precision errors in the
stats are contracted away by later iterations (simulated and measured:
~3.5e-4 max rel err, same as all-bf16). Step 9 (the last row step) is exact
fp32, fused into the final scaling pass via scalar_tensor_tensor with
accum_out=rowsum; row normalization self-corrects any surviving r error.

Matrices are processed in interleaved groups of GRP=5, emitting each pipeline
sub-phase (matvecs, psum->sbuf copies, tiny transposes, vector updates) for
all group members back-to-back so the in-order engines always have
independent work queued. PSUM tiles are statically pinned (one bank each,
counter-rotated): lazy PSUM bank reuse across kinds otherwise makes Tile emit
cross-tile WAW waits that stall matmuls. Tiny [1,512]->[128,4] transposes
land in unused columns of the same mv bank.

Built on bacc.Bacc (not bass.Bass): Bacc.compile() legalizes multi-wait
sync_info that this walrus rejects. Sharding: pure data parallel,
batch 256 -> 32 matrices per core x 8 cores.
"""

import sys

sys.path.insert(0, "/opt/trn_rl_repo")

import numpy as np

import concourse.bacc as bacc
import concourse.mybir as mybir
import concourse.tile as tile
from concourse import masks
from concourse.bass_utils import run_bass_kernel_spmd

N_CORES = 8
B_SHARD = 32  # 256 / 8
N = 512
P = 128
NCH = N // P  # 4 chunks of 128
NPAIR = NCH // 2  # 2 DoubleRow pairs
EPS = 1e-4
F32 = mybir.dt.float32
BF16 = mybir.dt.bfloat16
FP8 = mybir.dt.float8e4
I32 = mybir.dt.int32

MUL = mybir.AluOpType.mult
DIV = mybir.AluOpType.divide
DR = mybir.MatmulPerfMode.DoubleRow

USE_FP8 = True


def build_program(repeat=1):
    """repeat>1 wraps the whole body in a HW For_i loop (identical result
    each iteration) - used by bench.py to measure HW time via wall-clock
    slope between two repeat counts, since the axon RPC floor (~79ms)
    hides a single execution."""
    import contextlib

    nc = bacc.Bacc()
    s_in = nc.declare_dram_parameter("s", [B_SHARD, N, N], F32, isOutput=False)
    s_out = nc.declare_dram_parameter("out", [B_SHARD, N, N], F32, isOutput=True)

    with tile.TileContext(nc) as tc:
        with (
            tc.tile_pool(name="singles", bufs=1) as singles,
            tc.tile_pool(name="data", bufs=GRP + 1) as data,
            tc.tile_pool(name="vec", bufs=GRP + 2) as vec,
            tc.tile_pool(name="psum_fix", bufs=1, space="PSUM") as psum_fix,
        ):
            ident_t = singles.tile([P, P], BF16)
            masks.make_identity(nc, ident_t[:])
            one11 = singles.tile([1, 1], F32)
            nc.gpsimd.memset(one11[:], 1.0)
            ones_row = singles.tile([1, P], F32)
            nc.gpsimd.memset(ones_row[:], 1.0)

            # Statically pinned PSUM, one full bank per tile. Writers: PE.
            # Readers: mv/tr -> ACT only; stg/bc -> DVE only.
            mv5 = [
                psum_fix.tile([P, N], F32, tag=f"mv{i}", name=f"mv{i}")
                for i in range(5)
            ]
            stg2 = [
                psum_fix.tile([P, 2 * N], BF16, tag=f"stg{i}", name=f"stg{i}")
                for i in range(2)
            ]
            bc = psum_fix.tile([P, N], F32, tag="bc")

            consts = (ident_t, one11, ones_row)
            psum = (mv5, stg2, bc)
            loop_cm = (
                tc.For_i(0, repeat, 1) if repeat > 1 else contextlib.nullcontext()
            )
            with loop_cm:
                body(nc, tc, s_in, s_out, data, vec, consts, psum)
    nc.compile()
    return nc


GRP = 5  # matrices interleaved in flight; must be <= len(mv5)


def emit_build(nc, b, s_in, data, vec, consts, psum, cnt):
    """Load + casts + transposed fp8 copy for one matrix; returns tile dict."""
    ident_t, one11, ones_row = consts
    mv5, stg2, bc = psum

    sf = data.tile([P, NCH, N], F32, tag="sf", name="sf")
    dram_view = s_in[b].rearrange("(t p) n -> p t n", p=P)
    nc.sync.dma_start(sf[:], dram_view)

    sq8 = data.tile([P, NCH, N], FP8, tag="sq8", name="sq8")
    nc.gpsimd.tensor_copy(sq8[:], sf[:])
    sb = data.tile([P, NCH, N], BF16, tag="sb", name="sb")
    nc.vector.tensor_copy(sb[:], sf[:])

    st8 = data.tile([P, NCH, N], FP8, tag="st8", name="st8")
    for u in range(NCH):
        stg = stg2[cnt[1] % 2]
        cnt[1] += 1
        for t in range(NCH):
            nc.tensor.transpose(
                stg[:, t * P : (t + 1) * P],
                sb[:, t, u * P : (u + 1) * P],
                ident_t[:],
            )
        nc.vector.tensor_copy(st8[:, u, :], stg[:, :N])

    r = vec.tile([P, NCH], F32, tag="r", name="r")
    c = vec.tile([P, NCH], F32, tag="c", name="c")
    q = vec.tile([P, NCH], F32, tag="q", name="q")  # q = 1/r
    r8p = vec.tile([P, NPAIR, 2, 16], FP8, tag="r8p", name="r8p")
    c8p = vec.tile([P, NPAIR, 2, 16], FP8, tag="c8p", name="c8p")
    rb = vec.tile([P, NCH], BF16, tag="rb", name="rb")
    nc.gpsimd.memset(q[:], 1.0)
    nc.gpsimd.memset(r8p[:], 1.0)
    return dict(sf=sf, sq8=sq8, sb=sb, st8=st8, r=r, c=c, q=q,
                r8p=r8p, c8p=c8p, rb=rb)


def emit_step_mm(nc, M, step, mvt):
    mv = mvt[:1, :]
    if step == 8:
        for m in range(NCH):
            nc.tensor.matmul(
                mv, M["rb"][:, m : m + 1], M["sb"][:, m, :],
                start=(m == 0), stop=(m == NCH - 1),
            )
    else:
        w8 = M["r8p"] if step % 2 == 0 else M["c8p"]
        d8 = M["sq8"] if step % 2 == 0 else M["st8"]
        for u in range(NPAIR):
            nc.tensor.matmul(
                mv, w8[:, u, :, 0:1], d8[:, 2 * u : 2 * u + 2, :],
                start=(u == 0), stop=(u == NPAIR - 1), perf_mode=DR,
            )


def emit_final(nc, b, M, s_out, vec, consts, psum):
    ident_t, one11, ones_row = consts
    mv5, stg2, bc = psum
    sf = M["sf"]
    nc.tensor.matmul(bc[:], ones_row[:], M["c8row"][:], start=True, stop=True)
    w = vec.tile([P, NCH], F32, tag="w", name="w")
    for t in range(NCH):
        nc.vector.scalar_tensor_tensor(
            out=sf[:, t, :], in0=sf[:, t, :], scalar=M["r"][:, t : t + 1],
            in1=bc[:], op0=MUL, op1=MUL, accum_out=w[:, t : t + 1],
        )
    rr = vec.tile([P, NCH], F32, tag="rr", name="rr")
    nc.vector.tensor_scalar_add(rr[:], w[:], EPS)
    nc.vector.reciprocal(rr[:], rr[:])
    for t in range(NCH):
        nc.gpsimd.tensor_scalar_mul(sf[:, t, :], sf[:, t, :], rr[:, t : t + 1])
    out_view = s_out[b].rearrange("(t p) n -> p t n", p=P)
    nc.sync.dma_start(out_view, sf[:])


def body(nc, tc, s_in, s_out, data, vec, consts, psum):
    ident_t, one11, ones_row = consts
    mv5, stg2, bc = psum
    cnt = [0, 0]  # [mv slot counter, stg slot counter]
    for g0 in range(0, B_SHARD, GRP):
        bs = list(range(g0, min(g0 + GRP, B_SHARD)))
        mats = [emit_build(nc, b, s_in, data, vec, consts, psum, cnt)
                for b in bs]
        for step in range(9):
            slots = []
            # sub-phase major: all matvecs, all urow copies, all tiny
            # transposes, all uv copies, all updates - so no engine's
            # in-order stream head-of-line blocks another matrix
            for M in mats:
                mvt = mv5[cnt[0] % len(mv5)]
                cnt[0] += 1
                slots.append(mvt)
                emit_step_mm(nc, M, step, mvt)
            for M, mvt in zip(mats, slots):
                urow = vec.tile([1, N], F32, tag="urow", name="urow")
                M["urow"] = urow
                nc.scalar.copy(urow[:], mvt[:1, :])
            if step == 8:
                for M in mats:
                    c8row = vec.tile([1, N], F32, tag="c8row", name="c8row")
                    M["c8row"] = c8row
                    nc.vector.reciprocal(c8row[:], M["urow"][:])
                break
            for M, mvt in zip(mats, slots):
                for m in range(NCH):
                    nc.tensor.transpose(
                        mvt[:, 128 + m : 129 + m],
                        M["urow"][:, m * P : (m + 1) * P], one11[:],
                    )
            for M, mvt in zip(mats, slots):
                uv = vec.tile([P, NCH], F32, tag="uv", name="uv")
                M["uv"] = uv
                nc.scalar.copy(uv[:], mvt[:, 128 : 128 + NCH])
            for M in mats:
                uv, r, c, q = M["uv"], M["r"], M["c"], M["q"]
                if step % 2 == 0:  # col update (cancel form): c = 1/u
                    nc.vector.reciprocal(c[:], uv[:])
                    nc.vector.tensor_copy(
                        M["c8p"][:, :, :, 0],
                        c[:].rearrange("p (a b) -> p a b", a=NPAIR),
                    )
                else:  # row update: q = eps*q + v ; r = 1/q
                    nc.vector.scalar_tensor_tensor(
                        out=q[:], in0=q[:], scalar=EPS, in1=uv[:],
                        op0=MUL, op1=mybir.AluOpType.add,
                    )
                    nc.vector.reciprocal(r[:], q[:])
                    if step < 7:
                        nc.vector.tensor_copy(
                            M["r8p"][:, :, :, 0],
                            r[:].rearrange("p (a b) -> p a b", a=NPAIR),
                        )
                    else:
                        nc.vector.tensor_copy(M["rb"][:], r[:])
        for b, M in zip(bs, mats):
            emit_final(nc, b, M, s_out, vec, consts, psum)


_PROGRAM = None


def _get_program():
    global _PROGRAM
    if _PROGRAM is None:
        _PROGRAM = build_program()
    return _PROGRAM


def kernel(**inputs):
    s = np.asarray(inputs["s"], dtype=np.float32)
    assert s.shape == (N_CORES * B_SHARD, N, N), s.shape
    nc = _get_program()
    in_maps = [
        {"s": np.ascontiguousarray(s[i * B_SHARD : (i + 1) * B_SHARD])}
        for i in range(N_CORES)
    ]
    res = run_bass_kernel_spmd(nc, in_maps, core_ids=list(range(N_CORES)))
    out = np.concatenate([res.results[i]["out"] for i in range(N_CORES)], axis=0)
    return out.astype(np.float32)


if __name__ == "__main__":
    rng = np.random.default_rng(0)
    s = rng.random((N_CORES * B_SHARD, N, N), dtype=np.float32)
    o = kernel(s=s)
    print(o.shape, o.dtype)
